# revision 4
# baseline (speedup 1.0000x reference)
"""Trainium2 Bass kernel for nn_AttentionRefinementModule (deformable conv + sigmoid).

Data-parallel over batch: 8 samples -> 8 NeuronCores. ~640us (baseline 1339us).

Design (vs the v1 pair-row-gather baseline):
  - 4-corner single descriptor: per tap k, U_k = W_k^T x is computed at every
    2-padded grid position and stored row-pair duplicated in DRAM
    (Upp[q] = [U[q] | U[q+68]], 512 bf16 per row), so ONE SWDGE descriptor
    (elem 1024 bf16 = 2KB spanning rows q,q+1) fetches all 4 bilinear corners
    -> 36864 descriptors total (~9ns each on GpSimd under load).
  - 4 SWDGE queues, round-robin per gather; gathers are the only Pool-engine
    DMAs so the 8 DMASW sem lanes map to a fixed queue each (lane i%8 <->
    queue i%4) - a lane's semaphores must never change queues.
  - Per-k-pair software pipeline: P4 (U matmuls on PE + psum->bf16 copies on
    Scalar + A/B writes on Sync/Scalar) for pair p+1 overlaps gather
    descriptor-gen (GpSimd) and the weighted reduce (DVE) of pair p.
  - 2-pixel zero padding kills all bilinear validity masking: clamped
    out-of-image corners read guaranteed zeros.
  - bf16 end-to-end; corner weights pre-expanded x32 so the weighted-corner
    multiply meets the DVE 2x perf-mode rules (16-bit, innermost stride 1 on
    ALL operands: the weight AP is [0,8],[32,32],[1,32]).
  - corner-pair sums accumulated over k into per-group sacc (one 2x add per
    gather), finalized + sigmoid per group as soon as tap 8 lands (P6:
    PE transpose via bf16 PSUM, Scalar sigmoid+bias, Sync store).
"""

import numpy as np

import concourse.bass as bass
import concourse.mybir as mybir
from concourse import bacc
from concourse.tile import TileContext
from concourse.bass_utils import run_bass_kernel_spmd

B, C, H, W = 8, 256, 64, 64
HW = H * W
NK = 9
PW = W + 4           # 68: 2-pixel zero pad each side (kills validity masks)
NPAD = 4800          # xpad free size per channel-half
NT = HW // 128       # 32 interior position tiles
NT2 = 37             # padded-grid tiles (37*128 = 4736 >= 68*68)
NQ = NT2 * 128       # 4736 Upp rows per k
NG = 4               # gather groups of 8 tiles (1024 idxs) each
F32 = mybir.dt.float32
BF16 = mybir.dt.bfloat16
I32 = mybir.dt.int32
I16 = mybir.dt.int16

_CACHE = {}


def build_nc(debug=False):
    nc = bacc.Bacc(num_swdge_queues=4)

    x_d = nc.declare_dram_parameter("x", [C, HW], F32, isOutput=False)
    wofflhsT_d = nc.declare_dram_parameter("wofflhsT", [128, 2, NK, 18], BF16, isOutput=False)
    wmov_d = nc.declare_dram_parameter("wmov", [128, 2, NK * 256], BF16, isOutput=False)
    boff_d = nc.declare_dram_parameter("boff", [18, 1], F32, isOutput=False)
    bias_d = nc.declare_dram_parameter("bias", [128, 2], F32, isOutput=False)
    ident_d = nc.declare_dram_parameter("ident", [128, 128], F32, isOutput=False)
    cyk_d = nc.declare_dram_parameter("cyk", [128, NT * NK], F32, isOutput=False)
    cxk_d = nc.declare_dram_parameter("cxk", [128, NK], F32, isOutput=False)
    sel_d = nc.declare_dram_parameter("sel", [128, 8, 16], F32, isOutput=False)
    out_d = nc.declare_dram_parameter("out", [C, HW], F32, isOutput=True)

    with TileContext(nc) as tc:
        free_order = []
        free_fns = {}

        def single(name, shape, dt=F32):
            t, fr = tc.tile(shape, dt, name=name)
            free_fns[name] = fr
            free_order.append(name)
            return t

        with (
            tc.tile_pool(name="pstr", bufs=2, space="PSUM") as ps_tr,
            tc.tile_pool(name="psut", bufs=2, space="PSUM") as ps_ut,
            tc.tile_pool(name="ps6", bufs=2, space="PSUM") as ps_6,
            tc.tile_pool(name="dram", bufs=1, space="DRAM") as dpool,
        ):
            # one DRAM tensor per k-PAIR (taps 2i, 2i+1) so gathers can fire
            # per-pair while later pairs still compute
            PAIRS = [(0, 1), (2, 3), (4, 5), (6, 7), (8,)]
            upp5 = [dpool.tile([len(pr) * NQ * 512], BF16, name=f"upp{pi}")
                    for pi, pr in enumerate(PAIRS)]

            def upp_base(kk):
                return upp5[kk // 2].tensor, upp5[kk // 2].offset + (kk % 2) * NQ * 512

            # ---- persistent tiles ----
            wg3b = single("wg3b", [128, NK, NT, 4], BF16)
            # idxw2[c, k, g, tt*8 + a]: wrapped int16 idx (p = 16a + c)
            idxw2 = single("idxw2", [128, NK, NG, 64], I16)
            outT = single("outT", [128, NT, 256], BF16)
            ident = single("ident", [128, 128])
            nc.sync.dma_start(out=ident[:, :], in_=ident_d[:, :])
            identb = single("identb", [128, 128], BF16)
            nc.vector.tensor_copy(identb[:, :], ident[:, :])
            zpad = single("zpad", [128, 256], BF16)
            nc.vector.memset(zpad[:, :], 0.0)
            sacc = single("sacc", [128, NG, 8, 2, 256], BF16)
            woff_sb = single("woff_sb", [128, 2, NK, 18], BF16)
            nc.sync.dma_start(out=woff_sb[:, :, :, :], in_=wofflhsT_d[:, :, :, :])
            wmov_sb = single("wmov_sb", [128, 2, NK * 256], BF16)
            nc.sync.dma_start(out=wmov_sb[:, :, :], in_=wmov_d[:, :, :])
            boff_sb = single("boff_sb", [18, 1])
            nc.sync.dma_start(out=boff_sb[:, :], in_=boff_d[:, :])
            bias_sb = single("bias_sb", [128, 2])
            nc.sync.dma_start(out=bias_sb[:, :], in_=bias_d[:, :])
            xpad = single("xpad", [128, 2, NPAD], BF16)

            # ---- freeable constants ----
            wg3 = single("wg3", [128, NK, NT, 4])
            cyk = single("cyk", [128, NT * NK])
            nc.sync.dma_start(out=cyk[:, :], in_=cyk_d[:, :])
            cxk = single("cxk", [128, NK])
            nc.sync.dma_start(out=cxk[:, :], in_=cxk_d[:, :])
            sel = single("sel", [128, 8, 16])
            nc.sync.dma_start(out=sel[:, :, :], in_=sel_d[:, :, :])

            # ---- P0: padded bf16 input image (2-pixel zero pad) ----
            # load f32 via HWDGE + cast on scalar: keeps GpSimd free of
            # pool-DMAs so gather queue assignment can round-robin cleanly
            nc.vector.memset(xpad[:, :, :], 0.0)
            x_sb = single("x_sb", [128, 2, HW])
            for g in range(2):
                nc.sync.dma_start(out=x_sb[:, g, :],
                                  in_=x_d[g * 128:(g + 1) * 128, :])
                dst = bass.AP(xpad.tensor,
                              xpad.offset + g * NPAD + 2 * PW + 2,
                              [xpad.ap[0], [PW, H], [1, W]])
                nc.scalar.activation(
                    dst, x_sb[:, g, :].rearrange("c (h w) -> c h w", w=W),
                    mybir.ActivationFunctionType.Copy)
            free_fns.pop("x_sb")()
            free_order.remove("x_sb")

            # ---- P1: offset conv on the padded grid (flat shifted views) ----
            # conv output at image (h, w) accumulates at column u = h*68 + w;
            # tap (ty, tx) samples xpad[u + (1+ty)*68 + (1+tx)]
            NP1 = H * PW - 4     # 4348 columns cover u in [0, 63*68+63]
            offp_sb = single("offp_sb", [18, NP1])
            off_sb = single("off_sb", [18, HW])
            nch1 = (NP1 + 511) // 512
            for n in range(nch1):
                nn = min(512, NP1 - n * 512)
                psb = ps_ut.tile([128, 1024], F32, name="ps_ut_t")
                ps = psb[:18, :512]
                first = True
                for g in range(2):
                    for t in range(NK):
                        ty, tx = t // 3, t % 3
                        o0 = (1 + ty) * PW + 1 + tx + n * 512
                        rhs = xpad[:, g, o0:o0 + nn]
                        nc.tensor.matmul(ps[:, :nn], woff_sb[:, g, t, :], rhs,
                                         start=first,
                                         stop=(g == 1 and t == NK - 1))
                        first = False
                nc.vector.tensor_scalar(offp_sb[:, n * 512:n * 512 + nn],
                                        ps[:, :nn], boff_sb[:, :], None,
                                        mybir.AluOpType.add)
            nc.vector.tensor_copy(
                off_sb.rearrange("j (h w) -> j h w", w=W),
                bass.AP(offp_sb.tensor, offp_sb.offset,
                        [offp_sb.ap[0], [PW, H], [1, W]]))

            # ---- P2: transpose off -> offT [128(hw%128), 32(t), 18] ----
            offT = single("offT", [128, NT, 18])
            for t in range(NT):
                pst = ps_tr.tile([128, 128], F32, name="ps_tr_t")
                nc.tensor.transpose(pst[:, :18],
                                    off_sb[:, t * 128:(t + 1) * 128],
                                    ident[:18, :18])
                nc.scalar.activation(offT[:, t, :], pst[:, :18],
                                     mybir.ActivationFunctionType.Copy)

            # ---- P4 compute for one k-pair (tile loop + Upp writes) ----
            def emit_p4(pi, upool):
                pair = PAIRS[pi]
                npr = len(pair)
                sl0 = pair[0] * 256
                ut, uo = upp5[pi].tensor, upp5[pi].offset
                # zero-fill the 68 tail rows of col2 first (disjoint from A/B
                # writes; the sim materializes the whole gather view)
                for i in range(npr):
                    dstZ = bass.AP(ut,
                                   uo + i * NQ * 512 + (NQ - 68) * 512 + 256,
                                   [[512, 68], [1, 256]])
                    nc.scalar.dma_start(out=dstZ, in_=zpad[:68, :])
                for th in range(0, NT2, 2):
                    ntl = min(2, NT2 - th)   # tiles in this block (last: 1)
                    psu = ps_ut.tile([128, 1024], F32, name="ps_ut_t")
                    for tl in range(ntl):
                        for g in range(2):
                            xt = xpad[:, g, (th + tl) * 128:(th + tl + 1) * 128]
                            nc.tensor.matmul(
                                psu[:, tl * 512:tl * 512 + npr * 256], xt,
                                wmov_sb[:, g, sl0:sl0 + npr * 256],
                                start=(g == 0), stop=(g == 1))
                    utsb = upool.tile([128, 1024], BF16, name="utsb_t")
                    # pack [tile, k, col] contiguous; one activation per block
                    nc.scalar.activation(
                        utsb[:, :ntl * npr * 256],
                        bass.AP(psu.tensor, psu.offset,
                                [psu.ap[0], [512, ntl], [1, npr * 256]]),
                        mybir.ActivationFunctionType.Copy)
                    # dual-tap writes per tile for col1 (A), col2 (B)
                    for tl in range(ntl):
                        t = th + tl
                        src = utsb[:, tl * npr * 256:(tl + 1) * npr * 256]
                        dstA = bass.AP(ut, uo + t * 128 * 512,
                                       [[512, 128], [NQ * 512, npr], [1, 256]])
                        nc.sync.dma_start(out=dstA, in_=src)
                        if t == 0:
                            dstB = bass.AP(ut, uo + 256,
                                           [[512, 60], [NQ * 512, npr],
                                            [1, 256]])
                            nc.scalar.dma_start(out=dstB,
                                                in_=utsb[68:128, :npr * 256])
                        else:
                            dstB = bass.AP(ut,
                                           uo + (t * 128 - 68) * 512 + 256,
                                           [[512, 128], [NQ * 512, npr],
                                            [1, 256]])
                            nc.scalar.dma_start(out=dstB, in_=src)
            with tc.tile_pool(name="utsb0", bufs=2) as upool0:
                emit_p4(0, upool0)

            # ---- P3: corner weights + 4-corner gather indices ----
            # 2-pixel padding guarantees out-of-image corners read zeros, so
            # no validity masking is needed (|off| <= ~2.5 << pad margin + clamp)
            SH = [128, NT, NK]
            dyx = offT.rearrange("p t (k two) -> p two t k", two=2)
            dy, dx = dyx[:, 0], dyx[:, 1]

            def split16(d_ap, c_ap, nm):
                # p16 = d + (base + tap + 16); t2 = floor(p16), w1 = frac
                p16 = single(nm + "_p16", SH)
                nc.vector.tensor_add(p16[:, :, :], d_ap, c_ap)
                ti = single(nm + "_ti", SH, I32)
                nc.vector.tensor_copy(ti[:, :, :], p16[:, :, :])
                tif = single(nm + "_tif", SH)
                nc.vector.tensor_copy(tif[:, :, :], ti[:, :, :])
                fr = single(nm + "_fr", SH)
                nc.vector.tensor_sub(fr[:, :, :], p16[:, :, :], tif[:, :, :])
                ng = single(nm + "_ng", SH)
                nc.vector.tensor_scalar(ng[:, :, :], fr[:, :, :], 0.0, None,
                                        mybir.AluOpType.is_lt)
                w1 = single(nm + "_w1", SH)
                nc.vector.tensor_add(w1[:, :, :], fr[:, :, :], ng[:, :, :])
                t2 = single(nm + "_t2", SH)
                nc.vector.tensor_sub(t2[:, :, :], tif[:, :, :], ng[:, :, :])
                return t2, w1

            t2y, wy1 = split16(dy, cyk.rearrange("p (t k) -> p t k", k=NK),
                               "y")
            t2x, wx1 = split16(dx, bass.AP(cxk.tensor, cxk.offset,
                                           [cxk.ap[0], [0, NT], [1, NK]]),
                               "x")
            wy0 = single("wy0", SH)
            nc.vector.tensor_scalar(wy0[:, :, :], wy1[:, :, :], -1.0, 1.0,
                                    mybir.AluOpType.mult, mybir.AluOpType.add)
            wx0 = single("wx0", SH)
            nc.vector.tensor_scalar(wx0[:, :, :], wx1[:, :, :], -1.0, 1.0,
                                    mybir.AluOpType.mult, mybir.AluOpType.add)

            # corner order j = 2*xc + r matches elem [U[q] U[q+68] U[q+1] U[q+69]]
            def gkt_view(ap3):
                # [128, NT, NK] (strides t:NK, k:1) iterated as (k, t)
                return bass.AP(ap3.tensor, ap3.offset,
                               [ap3.ap[0], [1, NK], [NK, NT]])

            for r, wyc in enumerate([wy0, wy1]):
                for xc, wxc in enumerate([wx0, wx1]):
                    j = 2 * xc + r
                    wdst = bass.AP(wg3.tensor, wg3.offset + j,
                                   [wg3.ap[0], [NT * 4, NK], [4, NT]])
                    nc.vector.tensor_mul(wdst, gkt_view(wyc), gkt_view(wxc))
            nc.vector.tensor_copy(wg3b[:, :, :, :], wg3[:, :, :, :])

            # gather idx: q0 = clamp(y0+2,0,66)*68 + clamp(x0+2,0,66)
            # with t2 = floor(p)+16 -> clamp(t2-14, 0, 66)
            xq = single("xq", SH)
            nc.vector.tensor_scalar(xq[:, :, :], t2x[:, :, :], -14.0, 0.0,
                                    mybir.AluOpType.add, mybir.AluOpType.max)
            nc.vector.tensor_scalar_min(xq[:, :, :], xq[:, :, :], 66.0)
            yq = single("yq", SH)
            nc.vector.tensor_scalar(yq[:, :, :], t2y[:, :, :], -14.0, 0.0,
                                    mybir.AluOpType.add, mybir.AluOpType.max)
            nc.vector.tensor_scalar_min(yq[:, :, :], yq[:, :, :], 66.0)
            nc.vector.tensor_scalar_mul(yq[:, :, :], yq[:, :, :], float(PW))
            idxq = single("idxq", [128, NK, NT])
            nc.vector.tensor_add(
                idxq[:, :, :],
                yq.rearrange("p t k -> p k t"),
                xq.rearrange("p t k -> p k t"))

            # ---- P3b: fold idxq into wrapped int16 gather layout ----
            # gather j-th idx (j = tt*128 + p) -> wrapped (c=p%16, s=tt*8+p//16)
            for k in range(NK):
                for a in range(8):
                    psq = ps_tr.tile([128, 128], F32, name="ps_tr_t")
                    nc.tensor.matmul(psq[:16, :NT], sel[:, a, :],
                                     idxq[:, k, :], start=True, stop=True)
                    # src col t = g*8 + tt ; dst col = g*64 + tt*8 + a
                    dst = bass.AP(idxw2.tensor,
                                  idxw2.offset + k * (NG * 64) + a,
                                  [[idxw2.ap[0][0], 16], [64, NG], [8, 8]])
                    src = bass.AP(psq.tensor, psq.offset,
                                  [[psq.ap[0][0], 16], [8, NG], [1, 8]])
                    nc.scalar.activation(dst, src,
                                         mybir.ActivationFunctionType.Copy)
            idxw_flat = idxw2.rearrange("p k g c -> p (k g c)")
            for step in (16, 32, 64):
                nc.sync.dma_start(out=idxw_flat[step:2 * step, :],
                                  in_=idxw_flat[0:step, :])

            keep = {"wg3b", "idxw2", "outT", "ident", "identb", "zpad",
                    "sacc", "woff_sb", "wmov_sb", "boff_sb", "bias_sb", "xpad"}
            for nm in reversed(free_order):
                if nm not in keep:
                    free_fns.pop(nm)()

            # ---- P4 + P5 pipelined over k-pairs ----
            with (
                tc.tile_pool(name="utsb", bufs=3) as upool,
                tc.tile_pool(name="wxp", bufs=2) as wpool,
                tc.tile_pool(name="gat", bufs=3) as gpool,
                tc.tile_pool(name="gtm", bufs=2) as mpool,
                tc.tile_pool(name="red", bufs=2) as rpool,
                tc.tile_pool(name="osb", bufs=4) as obpool,
            ):
                # DMASW sem lanes rotate over Pool-engine DMAs (8 lanes); a
                # lane's semaphores lock to one SWDGE queue. Gathers are the
                # ONLY pool DMAs now, so lane i%8 <-> queue i%4 round-robins.
                qn = 0

                def emit_p6_grp(g):
                    # transpose back, bias+sigmoid, store for tiles of grp g
                    for tt in range(8):
                        t = 8 * g + tt
                        for hh in range(2):
                            pso = ps_6.tile([128, 128], BF16, name="ps6_t")
                            nc.tensor.transpose(
                                pso[:, :],
                                outT[:, t, hh * 128:(hh + 1) * 128],
                                identb[:, :])
                            osb = obpool.tile([128, 128], F32, name="osb_t")
                            nc.scalar.activation(
                                osb[:, :], pso[:, :],
                                mybir.ActivationFunctionType.Sigmoid,
                                bias=bias_sb[:, hh:hh + 1], scale=1.0)
                            nc.sync.dma_start(
                                out=out_d[hh * 128:(hh + 1) * 128,
                                          t * 128:(t + 1) * 128],
                                in_=osb[:, :])

                def emit_p5(pi):
                    nonlocal qn
                    for i, kk in enumerate(PAIRS[pi]):
                        ut, uo = upp_base(kk)
                        # weight expansion x32: wexp32[p, tj(128), 32]
                        wexp = wpool.tile([128, 128, 32], BF16, name="wexp_t")
                        nc.vector.tensor_copy(
                            bass.AP(wexp.tensor, wexp.offset,
                                    [wexp.ap[0], [32, 128], [1, 32]]),
                            bass.AP(wg3b.tensor, wg3b.offset + kk * 128,
                                    [wg3b.ap[0], [1, 128], [0, 32]]))
                        for g in range(NG):
                            gt = gpool.tile([128, 8, 1024], BF16, name="gt_t")
                            in_ap = bass.AP(ut, uo, [[512, NQ - 2], [1, 1024]])
                            # last tap: split calls so the tail pipeline
                            # (reduce+P6 per group) starts sooner
                            nsplit = 2 if kk == 8 else 1
                            for h in range(nsplit):
                                w = 8 // nsplit
                                nc.gpsimd.dma_gather(
                                    out_ap=gt[:, w * h:w * (h + 1), :],
                                    in_ap=in_ap,
                                    idxs_ap=idxw2[:, kk, g,
                                                  8 * w * h:8 * w * (h + 1)],
                                    num_idxs=128 * w, num_idxs_reg=128 * w,
                                    elem_size=1024, elem_step=512,
                                    queue_num=qn % 4)
                                qn += 1
                            # weighted corners: iterate (rep8, tj32, o32)
                            gtm = mpool.tile([128, 8192], BF16, name="gtm_t")
                            v_out = bass.AP(gtm.tensor, gtm.offset,
                                            [gtm.ap[0], [32, 8], [256, 32],
                                             [1, 32]])
                            v_in = bass.AP(gt.tensor, gt.offset,
                                           [gt.ap[0], [32, 8], [256, 32],
                                            [1, 32]])
                            v_w = bass.AP(wexp.tensor,
                                          wexp.offset + g * 1024,
                                          [wexp.ap[0], [0, 8], [32, 32],
                                           [1, 32]])
                            nc.vector.tensor_tensor(v_out, v_in, v_w,
                                                    op=mybir.AluOpType.mult)
                            # corner pair sums: s1[p, tt, r, 256]
                            s1 = rpool.tile([128, 8, 2, 256], BF16, name="s1_t")
                            ga = bass.AP(gtm.tensor, gtm.offset,
                                         [gtm.ap[0], [1024, 8], [256, 2],
                                          [1, 256]])
                            gb = bass.AP(gtm.tensor, gtm.offset + 512,
                                         [gtm.ap[0], [1024, 8], [256, 2],
                                          [1, 256]])
                            nc.vector.tensor_add(s1[:, :, :, :], ga, gb)
                            sg = sacc[:, g, :, :, :]
                            if kk == 0:
                                nc.vector.tensor_copy(sg, s1[:, :, :, :])
                            else:
                                nc.vector.tensor_add(sg, sg, s1[:, :, :, :])
                            if kk == 8:
                                dstT = outT[:, 8 * g:8 * (g + 1), :]
                                nc.vector.tensor_add(dstT,
                                                     sacc[:, g, :, 0, :],
                                                     sacc[:, g, :, 1, :])
                                emit_p6_grp(g)

                emit_p5(0)
                for pi in range(1, len(PAIRS)):
                    emit_p4(pi, upool)
                    emit_p5(pi)

            for nm in reversed(free_order):
                if nm in free_fns:
                    free_fns.pop(nm)()

    nc.compile()
    return nc


def prepack(w_off, b_off, w, b):
    import ml_dtypes
    wofflhsT = np.zeros((2, NK, 128, 18), np.float32)
    for g in range(2):
        for t in range(NK):
            ty, tx = t // 3, t % 3
            wofflhsT[g, t] = w_off[:, g * 128:(g + 1) * 128, ty, tx].T
    wmov = np.zeros((2, 128, NK * 256), np.float32)
    for g in range(2):
        for k in range(NK):
            ky, kx = k // 3, k % 3
            wmov[g, :, k * 256:(k + 1) * 256] = w[:, g * 128:(g + 1) * 128, ky, kx].T
    p = np.arange(128)
    basey = (p[:, None] // 64 + 2 * np.arange(NT)[None, :]).astype(np.float32)
    kk = np.arange(NK)
    ky = (kk // 3 - 1).astype(np.float32)
    kx = (kk % 3 - 1).astype(np.float32)
    cyk = (basey[:, :, None] + ky[None, None, :] + 16.0).reshape(128, NT * NK)
    cxk = ((p % 64).astype(np.float32)[:, None] + kx[None, :] + 16.0)
    sel = np.zeros((128, 8, 16), np.float32)
    for a in range(8):
        for bb in range(16):
            sel[16 * a + bb, a, bb] = 1.0
    return {
        "wofflhsT": np.ascontiguousarray(
            wofflhsT.transpose(2, 0, 1, 3)).astype(ml_dtypes.bfloat16),
        "wmov": np.ascontiguousarray(
            wmov.transpose(1, 0, 2)).astype(ml_dtypes.bfloat16),
        "boff": b_off.reshape(18, 1).astype(np.float32),
        "bias": np.stack([b[:128], b[128:]], axis=1).astype(np.float32).copy(),
        "ident": np.eye(128, dtype=np.float32),
        "cyk": np.ascontiguousarray(cyk),
        "cxk": np.ascontiguousarray(cxk),
        "sel": sel,
    }


def make_in_maps(Fstagei, w_off, b_off, w, b):
    shared = prepack(np.asarray(w_off), np.asarray(b_off), np.asarray(w),
                     np.asarray(b))
    in_maps = []
    for i in range(B):
        m = dict(shared)
        m["x"] = np.ascontiguousarray(
            np.asarray(Fstagei[i]).reshape(C, HW).astype(np.float32))
        in_maps.append(m)
    return in_maps


def kernel(Fstagei, w_off, b_off, w, b):
    if "nc" not in _CACHE:
        _CACHE["nc"] = build_nc()
    nc = _CACHE["nc"]
    in_maps = make_in_maps(Fstagei, w_off, b_off, w, b)
    res = run_bass_kernel_spmd(nc, in_maps, core_ids=list(range(B)), trace=False)
    out = np.stack([np.asarray(res.results[i]["out"]).reshape(C, H, W)
                    for i in range(B)])
    return out.astype(np.float32)


# revision 5
# speedup vs baseline: 1.0168x; 1.0168x over previous
"""Trainium2 Bass kernel for nn_AttentionRefinementModule (deformable conv + sigmoid).

Data-parallel over batch: 8 samples -> 8 NeuronCores. ~640us (baseline 1339us).

Design (vs the v1 pair-row-gather baseline):
  - 4-corner single descriptor: per tap k, U_k = W_k^T x is computed at every
    2-padded grid position and stored row-pair duplicated in DRAM
    (Upp[q] = [U[q] | U[q+68]], 512 bf16 per row), so ONE SWDGE descriptor
    (elem 1024 bf16 = 2KB spanning rows q,q+1) fetches all 4 bilinear corners
    -> 36864 descriptors total (~9ns each on GpSimd under load).
  - 4 SWDGE queues, round-robin per gather; gathers are the only Pool-engine
    DMAs so the 8 DMASW sem lanes map to a fixed queue each (lane i%8 <->
    queue i%4) - a lane's semaphores must never change queues.
  - Per-k-pair software pipeline: P4 (U matmuls on PE + psum->bf16 copies on
    Scalar + A/B writes on Sync/Scalar) for pair p+1 overlaps gather
    descriptor-gen (GpSimd) and the weighted reduce (DVE) of pair p.
  - 2-pixel zero padding kills all bilinear validity masking: clamped
    out-of-image corners read guaranteed zeros.
  - bf16 end-to-end; corner weights pre-expanded x32 so the weighted-corner
    multiply meets the DVE 2x perf-mode rules (16-bit, innermost stride 1 on
    ALL operands: the weight AP is [0,8],[32,32],[1,32]).
  - corner-pair sums accumulated over k into per-group sacc (one 2x add per
    gather), finalized + sigmoid per group as soon as tap 8 lands (P6:
    PE transpose via bf16 PSUM, Scalar sigmoid+bias, Sync store).
"""

import numpy as np

import concourse.bass as bass
import concourse.mybir as mybir
from concourse import bacc
from concourse.tile import TileContext
from concourse.bass_utils import run_bass_kernel_spmd

B, C, H, W = 8, 256, 64, 64
HW = H * W
NK = 9
PW = W + 4           # 68: 2-pixel zero pad each side (kills validity masks)
NPAD = 4800          # xpad free size per channel-half
NT = HW // 128       # 32 interior position tiles
NT2 = 37             # padded-grid tiles (37*128 = 4736 >= 68*68)
NQ = NT2 * 128       # 4736 Upp rows per k
NG = 4               # gather groups of 8 tiles (1024 idxs) each
F32 = mybir.dt.float32
BF16 = mybir.dt.bfloat16
I32 = mybir.dt.int32
I16 = mybir.dt.int16

_CACHE = {}


def build_nc(debug=False):
    nc = bacc.Bacc(num_swdge_queues=4)

    x_d = nc.declare_dram_parameter("x", [C, HW], F32, isOutput=False)
    wofflhsT_d = nc.declare_dram_parameter("wofflhsT", [128, 2, NK, 18], BF16, isOutput=False)
    wmov_d = nc.declare_dram_parameter("wmov", [128, 2, NK * 256], BF16, isOutput=False)
    boff_d = nc.declare_dram_parameter("boff", [18, 1], F32, isOutput=False)
    bias_d = nc.declare_dram_parameter("bias", [128, 2], F32, isOutput=False)
    ident_d = nc.declare_dram_parameter("ident", [128, 128], F32, isOutput=False)
    cyk_d = nc.declare_dram_parameter("cyk", [128, NT * NK], F32, isOutput=False)
    cxk_d = nc.declare_dram_parameter("cxk", [128, NK], F32, isOutput=False)
    sel_d = nc.declare_dram_parameter("sel", [128, 8, 16], F32, isOutput=False)
    out_d = nc.declare_dram_parameter("out", [C, HW], F32, isOutput=True)

    with TileContext(nc) as tc:
        free_order = []
        free_fns = {}

        def single(name, shape, dt=F32):
            t, fr = tc.tile(shape, dt, name=name)
            free_fns[name] = fr
            free_order.append(name)
            return t

        with (
            tc.tile_pool(name="pstr", bufs=2, space="PSUM") as ps_tr,
            tc.tile_pool(name="psut", bufs=2, space="PSUM") as ps_ut,
            tc.tile_pool(name="ps6", bufs=2, space="PSUM") as ps_6,
            tc.tile_pool(name="dram", bufs=1, space="DRAM") as dpool,
        ):
            # one DRAM tensor per k-PAIR (taps 2i, 2i+1) so gathers can fire
            # per-pair while later pairs still compute
            PAIRS = [(0, 1), (2, 3), (4, 5), (6, 7), (8,)]
            upp5 = [dpool.tile([len(pr) * NQ * 512], BF16, name=f"upp{pi}")
                    for pi, pr in enumerate(PAIRS)]

            def upp_base(kk):
                return upp5[kk // 2].tensor, upp5[kk // 2].offset + (kk % 2) * NQ * 512

            # ---- persistent tiles ----
            wg3b = single("wg3b", [128, NK, NT, 4], BF16)
            # idxw2[c, k, g, tt*8 + a]: wrapped int16 idx (p = 16a + c)
            idxw2 = single("idxw2", [128, NK, NG, 64], I16)
            outT = single("outT", [128, NT, 256], BF16)
            ident = single("ident", [128, 128])
            nc.sync.dma_start(out=ident[:, :], in_=ident_d[:, :])
            identb = single("identb", [128, 128], BF16)
            nc.vector.tensor_copy(identb[:, :], ident[:, :])
            zpad = single("zpad", [128, 256], BF16)
            nc.vector.memset(zpad[:, :], 0.0)
            sacc = single("sacc", [128, NG, 8, 2, 256], BF16)
            woff_sb = single("woff_sb", [128, 2, NK, 18], BF16)
            nc.sync.dma_start(out=woff_sb[:, :, :, :], in_=wofflhsT_d[:, :, :, :])
            wmov_sb = single("wmov_sb", [128, 2, NK * 256], BF16)
            nc.sync.dma_start(out=wmov_sb[:, :, :], in_=wmov_d[:, :, :])
            boff_sb = single("boff_sb", [18, 1])
            nc.sync.dma_start(out=boff_sb[:, :], in_=boff_d[:, :])
            bias_sb = single("bias_sb", [128, 2])
            nc.sync.dma_start(out=bias_sb[:, :], in_=bias_d[:, :])
            xpad = single("xpad", [128, 2, NPAD], BF16)

            # ---- freeable constants ----
            wg3 = single("wg3", [128, NK, NT, 4])
            cyk = single("cyk", [128, NT * NK])
            nc.sync.dma_start(out=cyk[:, :], in_=cyk_d[:, :])
            cxk = single("cxk", [128, NK])
            nc.sync.dma_start(out=cxk[:, :], in_=cxk_d[:, :])
            sel = single("sel", [128, 8, 16])
            nc.sync.dma_start(out=sel[:, :, :], in_=sel_d[:, :, :])

            # ---- P0: padded bf16 input image (2-pixel zero pad) ----
            # load f32 via HWDGE + cast on scalar: keeps GpSimd free of
            # pool-DMAs so gather queue assignment can round-robin cleanly
            nc.vector.memset(xpad[:, :, :], 0.0)
            x_sb = single("x_sb", [128, 2, HW])
            for g in range(2):
                nc.sync.dma_start(out=x_sb[:, g, :],
                                  in_=x_d[g * 128:(g + 1) * 128, :])
                dst = bass.AP(xpad.tensor,
                              xpad.offset + g * NPAD + 2 * PW + 2,
                              [xpad.ap[0], [PW, H], [1, W]])
                nc.scalar.activation(
                    dst, x_sb[:, g, :].rearrange("c (h w) -> c h w", w=W),
                    mybir.ActivationFunctionType.Copy)
            free_fns.pop("x_sb")()
            free_order.remove("x_sb")

            # ---- P1: offset conv on the padded grid (flat shifted views) ----
            # conv output at image (h, w) accumulates at column u = h*68 + w;
            # tap (ty, tx) samples xpad[u + (1+ty)*68 + (1+tx)]
            NP1 = H * PW - 4     # 4348 columns cover u in [0, 63*68+63]
            offp_sb = single("offp_sb", [18, NP1])
            off_sb = single("off_sb", [18, HW])
            nch1 = (NP1 + 511) // 512
            for n in range(nch1):
                nn = min(512, NP1 - n * 512)
                psb = ps_ut.tile([128, 1024], F32, name="ps_ut_t")
                ps = psb[:18, :512]
                first = True
                for g in range(2):
                    for t in range(NK):
                        ty, tx = t // 3, t % 3
                        o0 = (1 + ty) * PW + 1 + tx + n * 512
                        rhs = xpad[:, g, o0:o0 + nn]
                        nc.tensor.matmul(ps[:, :nn], woff_sb[:, g, t, :], rhs,
                                         start=first,
                                         stop=(g == 1 and t == NK - 1))
                        first = False
                nc.vector.tensor_scalar(offp_sb[:, n * 512:n * 512 + nn],
                                        ps[:, :nn], boff_sb[:, :], None,
                                        mybir.AluOpType.add)
            nc.vector.tensor_copy(
                off_sb.rearrange("j (h w) -> j h w", w=W),
                bass.AP(offp_sb.tensor, offp_sb.offset,
                        [offp_sb.ap[0], [PW, H], [1, W]]))

            # ---- P2: transpose off -> offT [128(hw%128), 32(t), 18] ----
            offT = single("offT", [128, NT, 18])
            for t in range(NT):
                pst = ps_tr.tile([128, 128], F32, name="ps_tr_t")
                nc.tensor.transpose(pst[:, :18],
                                    off_sb[:, t * 128:(t + 1) * 128],
                                    ident[:18, :18])
                nc.scalar.activation(offT[:, t, :], pst[:, :18],
                                     mybir.ActivationFunctionType.Copy)

            # ---- P4 compute for one k-pair (tile loop + Upp writes) ----
            def emit_p4(pi, upool):
                pair = PAIRS[pi]
                npr = len(pair)
                sl0 = pair[0] * 256
                ut, uo = upp5[pi].tensor, upp5[pi].offset
                # zero-fill the 68 tail rows of col2 first (disjoint from A/B
                # writes; the sim materializes the whole gather view)
                for i in range(npr):
                    dstZ = bass.AP(ut,
                                   uo + i * NQ * 512 + (NQ - 68) * 512 + 256,
                                   [[512, 68], [1, 256]])
                    nc.scalar.dma_start(out=dstZ, in_=zpad[:68, :])
                for th in range(0, NT2, 2):
                    ntl = min(2, NT2 - th)   # tiles in this block (last: 1)
                    psu = ps_ut.tile([128, 1024], F32, name="ps_ut_t")
                    for tl in range(ntl):
                        for g in range(2):
                            xt = xpad[:, g, (th + tl) * 128:(th + tl + 1) * 128]
                            nc.tensor.matmul(
                                psu[:, tl * 512:tl * 512 + npr * 256], xt,
                                wmov_sb[:, g, sl0:sl0 + npr * 256],
                                start=(g == 0), stop=(g == 1))
                    utsb = upool.tile([128, 1024], BF16, name="utsb_t")
                    # pack [tile, k, col] contiguous; one activation per block
                    nc.scalar.activation(
                        utsb[:, :ntl * npr * 256],
                        bass.AP(psu.tensor, psu.offset,
                                [psu.ap[0], [512, ntl], [1, npr * 256]]),
                        mybir.ActivationFunctionType.Copy)
                    # dual-tap writes per tile for col1 (A), col2 (B)
                    for tl in range(ntl):
                        t = th + tl
                        src = utsb[:, tl * npr * 256:(tl + 1) * npr * 256]
                        dstA = bass.AP(ut, uo + t * 128 * 512,
                                       [[512, 128], [NQ * 512, npr], [1, 256]])
                        nc.sync.dma_start(out=dstA, in_=src)
                        if t == 0:
                            dstB = bass.AP(ut, uo + 256,
                                           [[512, 60], [NQ * 512, npr],
                                            [1, 256]])
                            nc.scalar.dma_start(out=dstB,
                                                in_=utsb[68:128, :npr * 256])
                        else:
                            dstB = bass.AP(ut,
                                           uo + (t * 128 - 68) * 512 + 256,
                                           [[512, 128], [NQ * 512, npr],
                                            [1, 256]])
                            nc.scalar.dma_start(out=dstB, in_=src)
            with tc.tile_pool(name="utsb0", bufs=2) as upool0:
                emit_p4(0, upool0)

            # ---- P3: corner weights + 4-corner gather indices ----
            # 2-pixel padding guarantees out-of-image corners read zeros, so
            # no validity masking is needed (|off| <= ~2.5 << pad margin + clamp)
            SH = [128, NT, NK]
            dyx = offT.rearrange("p t (k two) -> p two t k", two=2)
            dy, dx = dyx[:, 0], dyx[:, 1]

            def split16(d_ap, c_ap, nm):
                # p16 = d + (base + tap + 16); t2 = floor(p16), w1 = frac
                p16 = single(nm + "_p16", SH)
                nc.vector.tensor_add(p16[:, :, :], d_ap, c_ap)
                ti = single(nm + "_ti", SH, I32)
                nc.vector.tensor_copy(ti[:, :, :], p16[:, :, :])
                tif = single(nm + "_tif", SH)
                nc.vector.tensor_copy(tif[:, :, :], ti[:, :, :])
                fr = single(nm + "_fr", SH)
                nc.vector.tensor_sub(fr[:, :, :], p16[:, :, :], tif[:, :, :])
                ng = single(nm + "_ng", SH)
                nc.vector.tensor_scalar(ng[:, :, :], fr[:, :, :], 0.0, None,
                                        mybir.AluOpType.is_lt)
                w1 = single(nm + "_w1", SH)
                nc.vector.tensor_add(w1[:, :, :], fr[:, :, :], ng[:, :, :])
                t2 = single(nm + "_t2", SH)
                nc.vector.tensor_sub(t2[:, :, :], tif[:, :, :], ng[:, :, :])
                return t2, w1

            t2y, wy1 = split16(dy, cyk.rearrange("p (t k) -> p t k", k=NK),
                               "y")
            t2x, wx1 = split16(dx, bass.AP(cxk.tensor, cxk.offset,
                                           [cxk.ap[0], [0, NT], [1, NK]]),
                               "x")
            wy0 = single("wy0", SH)
            nc.vector.tensor_scalar(wy0[:, :, :], wy1[:, :, :], -1.0, 1.0,
                                    mybir.AluOpType.mult, mybir.AluOpType.add)
            wx0 = single("wx0", SH)
            nc.vector.tensor_scalar(wx0[:, :, :], wx1[:, :, :], -1.0, 1.0,
                                    mybir.AluOpType.mult, mybir.AluOpType.add)

            # corner order j = 2*xc + r matches elem [U[q] U[q+68] U[q+1] U[q+69]]
            def gkt_view(ap3):
                # [128, NT, NK] (strides t:NK, k:1) iterated as (k, t)
                return bass.AP(ap3.tensor, ap3.offset,
                               [ap3.ap[0], [1, NK], [NK, NT]])

            for r, wyc in enumerate([wy0, wy1]):
                for xc, wxc in enumerate([wx0, wx1]):
                    j = 2 * xc + r
                    wdst = bass.AP(wg3.tensor, wg3.offset + j,
                                   [wg3.ap[0], [NT * 4, NK], [4, NT]])
                    nc.vector.tensor_mul(wdst, gkt_view(wyc), gkt_view(wxc))
            nc.vector.tensor_copy(wg3b[:, :, :, :], wg3[:, :, :, :])

            # gather idx: q0 = clamp(y0+2,0,66)*68 + clamp(x0+2,0,66)
            # with t2 = floor(p)+16 -> clamp(t2-14, 0, 66)
            xq = single("xq", SH)
            nc.vector.tensor_scalar(xq[:, :, :], t2x[:, :, :], -14.0, 0.0,
                                    mybir.AluOpType.add, mybir.AluOpType.max)
            nc.vector.tensor_scalar_min(xq[:, :, :], xq[:, :, :], 66.0)
            yq = single("yq", SH)
            nc.vector.tensor_scalar(yq[:, :, :], t2y[:, :, :], -14.0, 0.0,
                                    mybir.AluOpType.add, mybir.AluOpType.max)
            nc.vector.tensor_scalar_min(yq[:, :, :], yq[:, :, :], 66.0)
            nc.vector.tensor_scalar_mul(yq[:, :, :], yq[:, :, :], float(PW))
            idxq = single("idxq", [128, NK, NT])
            nc.vector.tensor_add(
                idxq[:, :, :],
                yq.rearrange("p t k -> p k t"),
                xq.rearrange("p t k -> p k t"))

            # ---- P3b: fold idxq into wrapped int16 gather layout ----
            # gather j-th idx (j = tt*128 + p) -> wrapped (c=p%16, s=tt*8+p//16)
            for k in range(NK):
                for a in range(8):
                    psq = ps_tr.tile([128, 128], F32, name="ps_tr_t")
                    nc.tensor.matmul(psq[:16, :NT], sel[:, a, :],
                                     idxq[:, k, :], start=True, stop=True)
                    # src col t = g*8 + tt ; dst col = g*64 + tt*8 + a
                    dst = bass.AP(idxw2.tensor,
                                  idxw2.offset + k * (NG * 64) + a,
                                  [[idxw2.ap[0][0], 16], [64, NG], [8, 8]])
                    src = bass.AP(psq.tensor, psq.offset,
                                  [[psq.ap[0][0], 16], [8, NG], [1, 8]])
                    nc.scalar.activation(dst, src,
                                         mybir.ActivationFunctionType.Copy)
            idxw_flat = idxw2.rearrange("p k g c -> p (k g c)")
            for step in (16, 32, 64):
                nc.sync.dma_start(out=idxw_flat[step:2 * step, :],
                                  in_=idxw_flat[0:step, :])

            keep = {"wg3b", "idxw2", "outT", "ident", "identb", "zpad",
                    "sacc", "woff_sb", "wmov_sb", "boff_sb", "bias_sb", "xpad"}
            for nm in reversed(free_order):
                if nm not in keep:
                    free_fns.pop(nm)()

            # ---- P4 + P5 pipelined over k-pairs ----
            with (
                tc.tile_pool(name="utsb", bufs=3) as upool,
                tc.tile_pool(name="wxp", bufs=2) as wpool,
                tc.tile_pool(name="gat", bufs=3) as gpool,
                tc.tile_pool(name="gtm", bufs=2) as mpool,
                tc.tile_pool(name="red", bufs=2) as rpool,
                tc.tile_pool(name="osb", bufs=4) as obpool,
            ):
                # DMASW sem lanes rotate over Pool-engine DMAs (8 lanes); a
                # lane's semaphores lock to one SWDGE queue. Gathers are the
                # ONLY pool DMAs now, so lane i%8 <-> queue i%4 round-robins.
                qn = 0

                def emit_p6_grp(t0, ntt):
                    # transpose back, bias+sigmoid, store for a tile range
                    for tt in range(ntt):
                        t = t0 + tt
                        for hh in range(2):
                            pso = ps_6.tile([128, 128], BF16, name="ps6_t")
                            nc.tensor.transpose(
                                pso[:, :],
                                outT[:, t, hh * 128:(hh + 1) * 128],
                                identb[:, :])
                            osb = obpool.tile([128, 128], F32, name="osb_t")
                            nc.scalar.activation(
                                osb[:, :], pso[:, :],
                                mybir.ActivationFunctionType.Sigmoid,
                                bias=bias_sb[:, hh:hh + 1], scale=1.0)
                            nc.sync.dma_start(
                                out=out_d[hh * 128:(hh + 1) * 128,
                                          t * 128:(t + 1) * 128],
                                in_=osb[:, :])

                def emit_p5(pi):
                    nonlocal qn
                    for i, kk in enumerate(PAIRS[pi]):
                        ut, uo = upp_base(kk)
                        # weight expansion x32: wexp32[p, tj(128), 32]
                        wexp = wpool.tile([128, 128, 32], BF16, name="wexp_t")
                        nc.vector.tensor_copy(
                            bass.AP(wexp.tensor, wexp.offset,
                                    [wexp.ap[0], [32, 128], [1, 32]]),
                            bass.AP(wg3b.tensor, wg3b.offset + kk * 128,
                                    [wg3b.ap[0], [1, 128], [0, 32]]))
                        for g in range(NG):
                            gt = gpool.tile([128, 8, 1024], BF16, name="gt_t")
                            gtm = mpool.tile([128, 8192], BF16, name="gtm_t")
                            s1 = rpool.tile([128, 8, 2, 256], BF16, name="s1_t")
                            in_ap = bass.AP(ut, uo, [[512, NQ - 2], [1, 1024]])
                            # last tap: half-group chunks so the tail
                            # (final add + P6) drains per 4 tiles
                            chunks = ((0, 4), (4, 4)) if kk == 8 else ((0, 8),)
                            for (t0, ntt) in chunks:
                                nc.gpsimd.dma_gather(
                                    out_ap=gt[:, t0:t0 + ntt, :],
                                    in_ap=in_ap,
                                    idxs_ap=idxw2[:, kk, g,
                                                  8 * t0:8 * (t0 + ntt)],
                                    num_idxs=128 * ntt, num_idxs_reg=128 * ntt,
                                    elem_size=1024, elem_step=512,
                                    queue_num=qn % 4)
                                qn += 1
                                co = t0 * 1024
                                # weighted corners: iterate (rep8, tj, o32)
                                v_out = bass.AP(gtm.tensor, gtm.offset + co,
                                                [gtm.ap[0], [32, 8],
                                                 [256, 4 * ntt], [1, 32]])
                                v_in = bass.AP(gt.tensor, gt.offset + co,
                                               [gt.ap[0], [32, 8],
                                                [256, 4 * ntt], [1, 32]])
                                v_w = bass.AP(wexp.tensor,
                                              wexp.offset + g * 1024 + t0 * 128,
                                              [wexp.ap[0], [0, 8],
                                               [32, 4 * ntt], [1, 32]])
                                nc.vector.tensor_tensor(v_out, v_in, v_w,
                                                        op=mybir.AluOpType.mult)
                                # corner pair sums: s1[p, tt, r, 256]
                                ga = bass.AP(gtm.tensor, gtm.offset + co,
                                             [gtm.ap[0], [1024, ntt], [256, 2],
                                              [1, 256]])
                                gb = bass.AP(gtm.tensor, gtm.offset + co + 512,
                                             [gtm.ap[0], [1024, ntt], [256, 2],
                                              [1, 256]])
                                nc.vector.tensor_add(s1[:, t0:t0 + ntt, :, :],
                                                     ga, gb)
                                sg = sacc[:, g, t0:t0 + ntt, :, :]
                                s1c = s1[:, t0:t0 + ntt, :, :]
                                if kk == 0:
                                    nc.vector.tensor_copy(sg, s1c)
                                else:
                                    nc.vector.tensor_add(sg, sg, s1c)
                                if kk == 8:
                                    dstT = outT[:, 8 * g + t0:
                                                8 * g + t0 + ntt, :]
                                    nc.vector.tensor_add(
                                        dstT,
                                        sacc[:, g, t0:t0 + ntt, 0, :],
                                        sacc[:, g, t0:t0 + ntt, 1, :])
                                    emit_p6_grp(8 * g + t0, ntt)

                emit_p5(0)
                for pi in range(1, len(PAIRS)):
                    emit_p4(pi, upool)
                    emit_p5(pi)

            for nm in reversed(free_order):
                if nm in free_fns:
                    free_fns.pop(nm)()

    nc.compile()
    return nc


def prepack(w_off, b_off, w, b):
    import ml_dtypes
    wofflhsT = np.zeros((2, NK, 128, 18), np.float32)
    for g in range(2):
        for t in range(NK):
            ty, tx = t // 3, t % 3
            wofflhsT[g, t] = w_off[:, g * 128:(g + 1) * 128, ty, tx].T
    wmov = np.zeros((2, 128, NK * 256), np.float32)
    for g in range(2):
        for k in range(NK):
            ky, kx = k // 3, k % 3
            wmov[g, :, k * 256:(k + 1) * 256] = w[:, g * 128:(g + 1) * 128, ky, kx].T
    p = np.arange(128)
    basey = (p[:, None] // 64 + 2 * np.arange(NT)[None, :]).astype(np.float32)
    kk = np.arange(NK)
    ky = (kk // 3 - 1).astype(np.float32)
    kx = (kk % 3 - 1).astype(np.float32)
    cyk = (basey[:, :, None] + ky[None, None, :] + 16.0).reshape(128, NT * NK)
    cxk = ((p % 64).astype(np.float32)[:, None] + kx[None, :] + 16.0)
    sel = np.zeros((128, 8, 16), np.float32)
    for a in range(8):
        for bb in range(16):
            sel[16 * a + bb, a, bb] = 1.0
    return {
        "wofflhsT": np.ascontiguousarray(
            wofflhsT.transpose(2, 0, 1, 3)).astype(ml_dtypes.bfloat16),
        "wmov": np.ascontiguousarray(
            wmov.transpose(1, 0, 2)).astype(ml_dtypes.bfloat16),
        "boff": b_off.reshape(18, 1).astype(np.float32),
        "bias": np.stack([b[:128], b[128:]], axis=1).astype(np.float32).copy(),
        "ident": np.eye(128, dtype=np.float32),
        "cyk": np.ascontiguousarray(cyk),
        "cxk": np.ascontiguousarray(cxk),
        "sel": sel,
    }


def make_in_maps(Fstagei, w_off, b_off, w, b):
    shared = prepack(np.asarray(w_off), np.asarray(b_off), np.asarray(w),
                     np.asarray(b))
    in_maps = []
    for i in range(B):
        m = dict(shared)
        m["x"] = np.ascontiguousarray(
            np.asarray(Fstagei[i]).reshape(C, HW).astype(np.float32))
        in_maps.append(m)
    return in_maps


def kernel(Fstagei, w_off, b_off, w, b):
    if "nc" not in _CACHE:
        _CACHE["nc"] = build_nc()
    nc = _CACHE["nc"]
    in_maps = make_in_maps(Fstagei, w_off, b_off, w, b)
    res = run_bass_kernel_spmd(nc, in_maps, core_ids=list(range(B)), trace=False)
    out = np.stack([np.asarray(res.results[i]["out"]).reshape(C, H, W)
                    for i in range(B)])
    return out.astype(np.float32)


# revision 6
# speedup vs baseline: 1.0439x; 1.0267x over previous
"""Trainium2 Bass kernel for nn_AttentionRefinementModule (deformable conv + sigmoid).

Data-parallel over batch: 8 samples -> 8 NeuronCores. ~640us (baseline 1339us).

Design (vs the v1 pair-row-gather baseline):
  - 4-corner single descriptor: per tap k, U_k = W_k^T x is computed at every
    2-padded grid position and stored row-pair duplicated in DRAM
    (Upp[q] = [U[q] | U[q+68]], 512 bf16 per row), so ONE SWDGE descriptor
    (elem 1024 bf16 = 2KB spanning rows q,q+1) fetches all 4 bilinear corners
    -> 36864 descriptors total (~9ns each on GpSimd under load).
  - 4 SWDGE queues, round-robin per gather; gathers are the only Pool-engine
    DMAs so the 8 DMASW sem lanes map to a fixed queue each (lane i%8 <->
    queue i%4) - a lane's semaphores must never change queues.
  - Per-k-pair software pipeline: P4 (U matmuls on PE + psum->bf16 copies on
    Scalar + A/B writes on Sync/Scalar) for pair p+1 overlaps gather
    descriptor-gen (GpSimd) and the weighted reduce (DVE) of pair p.
  - 2-pixel zero padding kills all bilinear validity masking: clamped
    out-of-image corners read guaranteed zeros.
  - bf16 end-to-end; corner weights pre-expanded x32 so the weighted-corner
    multiply meets the DVE 2x perf-mode rules (16-bit, innermost stride 1 on
    ALL operands: the weight AP is [0,8],[32,32],[1,32]).
  - corner-pair sums accumulated over k into per-group sacc (one 2x add per
    gather), finalized + sigmoid per group as soon as tap 8 lands (P6:
    PE transpose via bf16 PSUM, Scalar sigmoid+bias, Sync store).
"""

import numpy as np

import concourse.bass as bass
import concourse.mybir as mybir
from concourse import bacc
from concourse.tile import TileContext
from concourse.bass_utils import run_bass_kernel_spmd

B, C, H, W = 8, 256, 64, 64
HW = H * W
NK = 9
PW = W + 4           # 68: 2-pixel zero pad each side (kills validity masks)
NPAD = 4800          # xpad free size per channel-half
NT = HW // 128       # 32 interior position tiles
NT2 = 37             # padded-grid tiles (37*128 = 4736 >= 68*68)
NQ = NT2 * 128       # 4736 Upp rows per k
NG = 4               # gather groups of 8 tiles (1024 idxs) each
F32 = mybir.dt.float32
BF16 = mybir.dt.bfloat16
I32 = mybir.dt.int32
I16 = mybir.dt.int16

_CACHE = {}


def build_nc(debug=False):
    nc = bacc.Bacc(num_swdge_queues=4)

    x_d = nc.declare_dram_parameter("x", [C, HW], F32, isOutput=False)
    wofflhsT_d = nc.declare_dram_parameter("wofflhsT", [128, 2, NK, 18], BF16, isOutput=False)
    wmov_d = nc.declare_dram_parameter("wmov", [128, 2, NK * 256], BF16, isOutput=False)
    boff_d = nc.declare_dram_parameter("boff", [18, 1], F32, isOutput=False)
    bias_d = nc.declare_dram_parameter("bias", [128, 2], F32, isOutput=False)
    ident_d = nc.declare_dram_parameter("ident", [128, 128], F32, isOutput=False)
    cyk_d = nc.declare_dram_parameter("cyk", [128, NT * NK], F32, isOutput=False)
    cxk_d = nc.declare_dram_parameter("cxk", [128, NK], F32, isOutput=False)
    sel_d = nc.declare_dram_parameter("sel", [128, 8, 16], F32, isOutput=False)
    out_d = nc.declare_dram_parameter("out", [C, HW], F32, isOutput=True)

    with TileContext(nc) as tc:
        free_order = []
        free_fns = {}

        def single(name, shape, dt=F32):
            t, fr = tc.tile(shape, dt, name=name)
            free_fns[name] = fr
            free_order.append(name)
            return t

        with (
            tc.tile_pool(name="pstr", bufs=2, space="PSUM") as ps_tr,
            tc.tile_pool(name="psut", bufs=2, space="PSUM") as ps_ut,
            tc.tile_pool(name="ps6", bufs=2, space="PSUM") as ps_6,
            tc.tile_pool(name="dram", bufs=1, space="DRAM") as dpool,
        ):
            # one DRAM tensor per k-PAIR (taps 2i, 2i+1) so gathers can fire
            # per-pair while later pairs still compute
            PAIRS = [(0, 1), (2, 3), (4, 5), (6, 7), (8,)]
            upp5 = [dpool.tile([len(pr) * NQ * 512], BF16, name=f"upp{pi}")
                    for pi, pr in enumerate(PAIRS)]

            def upp_base(kk):
                return upp5[kk // 2].tensor, upp5[kk // 2].offset + (kk % 2) * NQ * 512

            # ---- persistent tiles ----
            wg3b = single("wg3b", [128, NK, NT, 4], BF16)
            # idxw2[c, k, g, tt*8 + a]: wrapped int16 idx (p = 16a + c)
            idxw2 = single("idxw2", [128, NK, NG, 64], I16)
            outT = single("outT", [128, NT, 256], BF16)
            ident = single("ident", [128, 128])
            nc.sync.dma_start(out=ident[:, :], in_=ident_d[:, :])
            identb = single("identb", [128, 128], BF16)
            nc.vector.tensor_copy(identb[:, :], ident[:, :])
            zpad = single("zpad", [128, 256], BF16)
            nc.vector.memset(zpad[:, :], 0.0)
            sacc = single("sacc", [128, NG, 8, 2, 256], BF16)
            woff_sb = single("woff_sb", [128, 2, NK, 18], BF16)
            nc.sync.dma_start(out=woff_sb[:, :, :, :], in_=wofflhsT_d[:, :, :, :])
            wmov_sb = single("wmov_sb", [128, 2, NK * 256], BF16)
            nc.sync.dma_start(out=wmov_sb[:, :, :], in_=wmov_d[:, :, :])
            boff_sb = single("boff_sb", [18, 1])
            nc.sync.dma_start(out=boff_sb[:, :], in_=boff_d[:, :])
            bias_sb = single("bias_sb", [128, 2])
            nc.sync.dma_start(out=bias_sb[:, :], in_=bias_d[:, :])
            xpad = single("xpad", [128, 2, NPAD], BF16)

            # ---- freeable constants ----
            wg3 = single("wg3", [128, NK, NT, 4])
            cyk = single("cyk", [128, NT * NK])
            nc.sync.dma_start(out=cyk[:, :], in_=cyk_d[:, :])
            cxk = single("cxk", [128, NK])
            nc.sync.dma_start(out=cxk[:, :], in_=cxk_d[:, :])
            sel = single("sel", [128, 8, 16])
            nc.sync.dma_start(out=sel[:, :, :], in_=sel_d[:, :, :])

            # ---- P0: padded bf16 input image (2-pixel zero pad) ----
            # load f32 via HWDGE + cast on scalar: keeps GpSimd free of
            # pool-DMAs so gather queue assignment can round-robin cleanly
            nc.vector.memset(xpad[:, :, :], 0.0)
            x_sb = single("x_sb", [128, 2, HW])
            for g in range(2):
                nc.sync.dma_start(out=x_sb[:, g, :],
                                  in_=x_d[g * 128:(g + 1) * 128, :])
                dst = bass.AP(xpad.tensor,
                              xpad.offset + g * NPAD + 2 * PW + 2,
                              [xpad.ap[0], [PW, H], [1, W]])
                nc.scalar.activation(
                    dst, x_sb[:, g, :].rearrange("c (h w) -> c h w", w=W),
                    mybir.ActivationFunctionType.Copy)
            free_fns.pop("x_sb")()
            free_order.remove("x_sb")

            # ---- P1: offset conv on the padded grid (flat shifted views) ----
            # conv output at image (h, w) accumulates at column u = h*68 + w;
            # tap (ty, tx) samples xpad[u + (1+ty)*68 + (1+tx)]
            NP1 = H * PW - 4     # 4348 columns cover u in [0, 63*68+63]
            offp_sb = single("offp_sb", [18, NP1])
            off_sb = single("off_sb", [18, HW])
            nch1 = (NP1 + 511) // 512
            for n in range(nch1):
                nn = min(512, NP1 - n * 512)
                psb = ps_ut.tile([128, 1024], F32, name="ps_ut_t")
                ps = psb[:18, :512]
                first = True
                for g in range(2):
                    for t in range(NK):
                        ty, tx = t // 3, t % 3
                        o0 = (1 + ty) * PW + 1 + tx + n * 512
                        rhs = xpad[:, g, o0:o0 + nn]
                        nc.tensor.matmul(ps[:, :nn], woff_sb[:, g, t, :], rhs,
                                         start=first,
                                         stop=(g == 1 and t == NK - 1))
                        first = False
                nc.vector.tensor_scalar(offp_sb[:, n * 512:n * 512 + nn],
                                        ps[:, :nn], boff_sb[:, :], None,
                                        mybir.AluOpType.add)
            nc.vector.tensor_copy(
                off_sb.rearrange("j (h w) -> j h w", w=W),
                bass.AP(offp_sb.tensor, offp_sb.offset,
                        [offp_sb.ap[0], [PW, H], [1, W]]))

            # ---- P2: transpose off -> offT [128(hw%128), 32(t), 18] ----
            offT = single("offT", [128, NT, 18])
            for t in range(NT):
                pst = ps_tr.tile([128, 128], F32, name="ps_tr_t")
                nc.tensor.transpose(pst[:, :18],
                                    off_sb[:, t * 128:(t + 1) * 128],
                                    ident[:18, :18])
                nc.scalar.activation(offT[:, t, :], pst[:, :18],
                                     mybir.ActivationFunctionType.Copy)

            # ---- P4 compute for one k-pair (tile loop + Upp writes) ----
            def emit_p4(pi, upool):
                pair = PAIRS[pi]
                npr = len(pair)
                sl0 = pair[0] * 256
                ut, uo = upp5[pi].tensor, upp5[pi].offset
                # zero-fill the 68 tail rows of col2 first (disjoint from A/B
                # writes; the sim materializes the whole gather view)
                for i in range(npr):
                    dstZ = bass.AP(ut,
                                   uo + i * NQ * 512 + (NQ - 68) * 512 + 256,
                                   [[512, 68], [1, 256]])
                    nc.scalar.dma_start(out=dstZ, in_=zpad[:68, :])
                for th in range(0, NT2, 2):
                    ntl = min(2, NT2 - th)   # tiles in this block (last: 1)
                    psu = ps_ut.tile([128, 1024], F32, name="ps_ut_t")
                    for tl in range(ntl):
                        for g in range(2):
                            xt = xpad[:, g, (th + tl) * 128:(th + tl + 1) * 128]
                            nc.tensor.matmul(
                                psu[:, tl * 512:tl * 512 + npr * 256], xt,
                                wmov_sb[:, g, sl0:sl0 + npr * 256],
                                start=(g == 0), stop=(g == 1))
                    utsb = upool.tile([128, 1024], BF16, name="utsb_t")
                    # pack [tile, k, col] contiguous; one activation per block
                    nc.scalar.activation(
                        utsb[:, :ntl * npr * 256],
                        bass.AP(psu.tensor, psu.offset,
                                [psu.ap[0], [512, ntl], [1, npr * 256]]),
                        mybir.ActivationFunctionType.Copy)
                    # dual-tap writes per tile for col1 (A), col2 (B)
                    for tl in range(ntl):
                        t = th + tl
                        src = utsb[:, tl * npr * 256:(tl + 1) * npr * 256]
                        dstA = bass.AP(ut, uo + t * 128 * 512,
                                       [[512, 128], [NQ * 512, npr], [1, 256]])
                        nc.sync.dma_start(out=dstA, in_=src)
                        if t == 0:
                            dstB = bass.AP(ut, uo + 256,
                                           [[512, 60], [NQ * 512, npr],
                                            [1, 256]])
                            nc.scalar.dma_start(out=dstB,
                                                in_=utsb[68:128, :npr * 256])
                        else:
                            dstB = bass.AP(ut,
                                           uo + (t * 128 - 68) * 512 + 256,
                                           [[512, 128], [NQ * 512, npr],
                                            [1, 256]])
                            nc.scalar.dma_start(out=dstB, in_=src)
            with tc.tile_pool(name="utsb0", bufs=2) as upool0:
                emit_p4(0, upool0)

            # ---- P3: corner weights + 4-corner gather indices ----
            # 2-pixel padding guarantees out-of-image corners read zeros, so
            # no validity masking is needed (|off| <= ~2.5 << pad margin + clamp)
            SH = [128, NT, NK]
            dyx = offT.rearrange("p t (k two) -> p two t k", two=2)
            dy, dx = dyx[:, 0], dyx[:, 1]

            def split16(d_ap, c_ap, nm):
                # p16 = d + (base + tap + 16); t2 = floor(p16), w1 = frac
                p16 = single(nm + "_p16", SH)
                nc.vector.tensor_add(p16[:, :, :], d_ap, c_ap)
                ti = single(nm + "_ti", SH, I32)
                nc.vector.tensor_copy(ti[:, :, :], p16[:, :, :])
                tif = single(nm + "_tif", SH)
                nc.vector.tensor_copy(tif[:, :, :], ti[:, :, :])
                fr = single(nm + "_fr", SH)
                nc.vector.tensor_sub(fr[:, :, :], p16[:, :, :], tif[:, :, :])
                ng = single(nm + "_ng", SH)
                nc.vector.tensor_scalar(ng[:, :, :], fr[:, :, :], 0.0, None,
                                        mybir.AluOpType.is_lt)
                w1 = single(nm + "_w1", SH)
                nc.vector.tensor_add(w1[:, :, :], fr[:, :, :], ng[:, :, :])
                t2 = single(nm + "_t2", SH)
                nc.vector.tensor_sub(t2[:, :, :], tif[:, :, :], ng[:, :, :])
                return t2, w1

            t2y, wy1 = split16(dy, cyk.rearrange("p (t k) -> p t k", k=NK),
                               "y")
            t2x, wx1 = split16(dx, bass.AP(cxk.tensor, cxk.offset,
                                           [cxk.ap[0], [0, NT], [1, NK]]),
                               "x")
            wy0 = single("wy0", SH)
            nc.vector.tensor_scalar(wy0[:, :, :], wy1[:, :, :], -1.0, 1.0,
                                    mybir.AluOpType.mult, mybir.AluOpType.add)
            wx0 = single("wx0", SH)
            nc.vector.tensor_scalar(wx0[:, :, :], wx1[:, :, :], -1.0, 1.0,
                                    mybir.AluOpType.mult, mybir.AluOpType.add)

            # corner order j = 2*xc + r matches elem [U[q] U[q+68] U[q+1] U[q+69]]
            def gkt_view(ap3):
                # [128, NT, NK] (strides t:NK, k:1) iterated as (k, t)
                return bass.AP(ap3.tensor, ap3.offset,
                               [ap3.ap[0], [1, NK], [NK, NT]])

            for r, wyc in enumerate([wy0, wy1]):
                for xc, wxc in enumerate([wx0, wx1]):
                    j = 2 * xc + r
                    wdst = bass.AP(wg3.tensor, wg3.offset + j,
                                   [wg3.ap[0], [NT * 4, NK], [4, NT]])
                    nc.vector.tensor_mul(wdst, gkt_view(wyc), gkt_view(wxc))
            nc.vector.tensor_copy(wg3b[:, :, :, :], wg3[:, :, :, :])

            # gather idx: q0 = clamp(y0+2,0,66)*68 + clamp(x0+2,0,66)
            # with t2 = floor(p)+16 -> clamp(t2-14, 0, 66)
            xq = single("xq", SH)
            nc.vector.tensor_scalar(xq[:, :, :], t2x[:, :, :], -14.0, 0.0,
                                    mybir.AluOpType.add, mybir.AluOpType.max)
            nc.vector.tensor_scalar_min(xq[:, :, :], xq[:, :, :], 66.0)
            yq = single("yq", SH)
            nc.vector.tensor_scalar(yq[:, :, :], t2y[:, :, :], -14.0, 0.0,
                                    mybir.AluOpType.add, mybir.AluOpType.max)
            nc.vector.tensor_scalar_min(yq[:, :, :], yq[:, :, :], 66.0)
            nc.vector.tensor_scalar_mul(yq[:, :, :], yq[:, :, :], float(PW))
            idxq = single("idxq", [128, NK, NT])
            nc.vector.tensor_add(
                idxq[:, :, :],
                yq.rearrange("p t k -> p k t"),
                xq.rearrange("p t k -> p k t"))

            # ---- P3b: fold idxq into wrapped int16 gather layout ----
            # gather j-th idx (j = tt*128 + p) -> wrapped (c=p%16, s=tt*8+p//16)
            for k in range(NK):
                for a in range(8):
                    psq = ps_tr.tile([128, 128], F32, name="ps_tr_t")
                    nc.tensor.matmul(psq[:16, :NT], sel[:, a, :],
                                     idxq[:, k, :], start=True, stop=True)
                    # src col t = g*8 + tt ; dst col = g*64 + tt*8 + a
                    dst = bass.AP(idxw2.tensor,
                                  idxw2.offset + k * (NG * 64) + a,
                                  [[idxw2.ap[0][0], 16], [64, NG], [8, 8]])
                    src = bass.AP(psq.tensor, psq.offset,
                                  [[psq.ap[0][0], 16], [8, NG], [1, 8]])
                    nc.scalar.activation(dst, src,
                                         mybir.ActivationFunctionType.Copy)
            idxw_flat = idxw2.rearrange("p k g c -> p (k g c)")
            for step in (16, 32, 64):
                nc.sync.dma_start(out=idxw_flat[step:2 * step, :],
                                  in_=idxw_flat[0:step, :])

            keep = {"wg3b", "idxw2", "outT", "ident", "identb", "zpad",
                    "sacc", "woff_sb", "wmov_sb", "boff_sb", "bias_sb", "xpad"}
            for nm in reversed(free_order):
                if nm not in keep:
                    free_fns.pop(nm)()

            # ---- P4 + P5 pipelined over k-pairs ----
            with (
                tc.tile_pool(name="utsb", bufs=3) as upool,
                tc.tile_pool(name="wxp", bufs=2) as wpool,
                tc.tile_pool(name="gat", bufs=5) as gpool,
                tc.tile_pool(name="gtm", bufs=2) as mpool,
                tc.tile_pool(name="red", bufs=2) as rpool,
                tc.tile_pool(name="osb", bufs=4) as obpool,
            ):
                # DMASW sem lanes rotate over Pool-engine DMAs (8 lanes); a
                # lane's semaphores lock to one SWDGE queue. Gathers are the
                # ONLY pool DMAs now, so lane i%8 <-> queue i%4 round-robins.
                qn = 0

                def emit_p6_grp(t0, ntt):
                    # transpose back, bias+sigmoid, store for a tile range
                    for tt in range(ntt):
                        t = t0 + tt
                        for hh in range(2):
                            pso = ps_6.tile([128, 128], BF16, name="ps6_t")
                            nc.tensor.transpose(
                                pso[:, :],
                                outT[:, t, hh * 128:(hh + 1) * 128],
                                identb[:, :])
                            osb = obpool.tile([128, 128], F32, name="osb_t")
                            nc.scalar.activation(
                                osb[:, :], pso[:, :],
                                mybir.ActivationFunctionType.Sigmoid,
                                bias=bias_sb[:, hh:hh + 1], scale=1.0)
                            nc.sync.dma_start(
                                out=out_d[hh * 128:(hh + 1) * 128,
                                          t * 128:(t + 1) * 128],
                                in_=osb[:, :])

                def emit_p5(pi):
                    nonlocal qn
                    for i, kk in enumerate(PAIRS[pi]):
                        ut, uo = upp_base(kk)
                        # weight expansion x32: wexp32[p, tj(128), 32]
                        wexp = wpool.tile([128, 128, 32], BF16, name="wexp_t")
                        nc.vector.tensor_copy(
                            bass.AP(wexp.tensor, wexp.offset,
                                    [wexp.ap[0], [32, 128], [1, 32]]),
                            bass.AP(wg3b.tensor, wg3b.offset + kk * 128,
                                    [wg3b.ap[0], [1, 128], [0, 32]]))
                        for g in range(NG):
                            gt = gpool.tile([128, 8, 1024], BF16, name="gt_t")
                            gtm = gt  # in-place weighted multiply
                            s1 = rpool.tile([128, 8, 2, 256], BF16, name="s1_t")
                            in_ap = bass.AP(ut, uo, [[512, NQ - 2], [1, 1024]])
                            # last tap: half-group chunks so the tail
                            # (final add + P6) drains per 4 tiles
                            chunks = ((0, 4), (4, 4)) if kk == 8 else ((0, 8),)
                            for (t0, ntt) in chunks:
                                nc.gpsimd.dma_gather(
                                    out_ap=gt[:, t0:t0 + ntt, :],
                                    in_ap=in_ap,
                                    idxs_ap=idxw2[:, kk, g,
                                                  8 * t0:8 * (t0 + ntt)],
                                    num_idxs=128 * ntt, num_idxs_reg=128 * ntt,
                                    elem_size=1024, elem_step=512,
                                    queue_num=qn % 4)
                                qn += 1
                                co = t0 * 1024
                                # weighted corners: iterate (rep8, tj, o32)
                                v_out = bass.AP(gtm.tensor, gtm.offset + co,
                                                [gtm.ap[0], [32, 8],
                                                 [256, 4 * ntt], [1, 32]])
                                v_in = bass.AP(gt.tensor, gt.offset + co,
                                               [gt.ap[0], [32, 8],
                                                [256, 4 * ntt], [1, 32]])
                                v_w = bass.AP(wexp.tensor,
                                              wexp.offset + g * 1024 + t0 * 128,
                                              [wexp.ap[0], [0, 8],
                                               [32, 4 * ntt], [1, 32]])
                                nc.vector.tensor_tensor(v_out, v_in, v_w,
                                                        op=mybir.AluOpType.mult)
                                # corner pair sums: s1[p, tt, r, 256]
                                ga = bass.AP(gtm.tensor, gtm.offset + co,
                                             [gtm.ap[0], [1024, ntt], [256, 2],
                                              [1, 256]])
                                gb = bass.AP(gtm.tensor, gtm.offset + co + 512,
                                             [gtm.ap[0], [1024, ntt], [256, 2],
                                              [1, 256]])
                                nc.vector.tensor_add(s1[:, t0:t0 + ntt, :, :],
                                                     ga, gb)
                                sg = sacc[:, g, t0:t0 + ntt, :, :]
                                s1c = s1[:, t0:t0 + ntt, :, :]
                                if kk == 0:
                                    nc.vector.tensor_copy(sg, s1c)
                                else:
                                    nc.vector.tensor_add(sg, sg, s1c)
                                if kk == 8:
                                    dstT = outT[:, 8 * g + t0:
                                                8 * g + t0 + ntt, :]
                                    nc.vector.tensor_add(
                                        dstT,
                                        sacc[:, g, t0:t0 + ntt, 0, :],
                                        sacc[:, g, t0:t0 + ntt, 1, :])
                                    emit_p6_grp(8 * g + t0, ntt)

                emit_p5(0)
                for pi in range(1, len(PAIRS)):
                    emit_p4(pi, upool)
                    emit_p5(pi)

            for nm in reversed(free_order):
                if nm in free_fns:
                    free_fns.pop(nm)()

    nc.compile()
    return nc


def prepack(w_off, b_off, w, b):
    import ml_dtypes
    wofflhsT = np.zeros((2, NK, 128, 18), np.float32)
    for g in range(2):
        for t in range(NK):
            ty, tx = t // 3, t % 3
            wofflhsT[g, t] = w_off[:, g * 128:(g + 1) * 128, ty, tx].T
    wmov = np.zeros((2, 128, NK * 256), np.float32)
    for g in range(2):
        for k in range(NK):
            ky, kx = k // 3, k % 3
            wmov[g, :, k * 256:(k + 1) * 256] = w[:, g * 128:(g + 1) * 128, ky, kx].T
    p = np.arange(128)
    basey = (p[:, None] // 64 + 2 * np.arange(NT)[None, :]).astype(np.float32)
    kk = np.arange(NK)
    ky = (kk // 3 - 1).astype(np.float32)
    kx = (kk % 3 - 1).astype(np.float32)
    cyk = (basey[:, :, None] + ky[None, None, :] + 16.0).reshape(128, NT * NK)
    cxk = ((p % 64).astype(np.float32)[:, None] + kx[None, :] + 16.0)
    sel = np.zeros((128, 8, 16), np.float32)
    for a in range(8):
        for bb in range(16):
            sel[16 * a + bb, a, bb] = 1.0
    return {
        "wofflhsT": np.ascontiguousarray(
            wofflhsT.transpose(2, 0, 1, 3)).astype(ml_dtypes.bfloat16),
        "wmov": np.ascontiguousarray(
            wmov.transpose(1, 0, 2)).astype(ml_dtypes.bfloat16),
        "boff": b_off.reshape(18, 1).astype(np.float32),
        "bias": np.stack([b[:128], b[128:]], axis=1).astype(np.float32).copy(),
        "ident": np.eye(128, dtype=np.float32),
        "cyk": np.ascontiguousarray(cyk),
        "cxk": np.ascontiguousarray(cxk),
        "sel": sel,
    }


def make_in_maps(Fstagei, w_off, b_off, w, b):
    shared = prepack(np.asarray(w_off), np.asarray(b_off), np.asarray(w),
                     np.asarray(b))
    in_maps = []
    for i in range(B):
        m = dict(shared)
        m["x"] = np.ascontiguousarray(
            np.asarray(Fstagei[i]).reshape(C, HW).astype(np.float32))
        in_maps.append(m)
    return in_maps


def kernel(Fstagei, w_off, b_off, w, b):
    if "nc" not in _CACHE:
        _CACHE["nc"] = build_nc()
    nc = _CACHE["nc"]
    in_maps = make_in_maps(Fstagei, w_off, b_off, w, b)
    res = run_bass_kernel_spmd(nc, in_maps, core_ids=list(range(B)), trace=False)
    out = np.stack([np.asarray(res.results[i]["out"]).reshape(C, H, W)
                    for i in range(B)])
    return out.astype(np.float32)
